# revision 40
# baseline (speedup 1.0000x reference)
"""DisagreementRegularizer Trainium2 kernel.

reference math:
    xn = x / max(||x||_2 along d, eps)
    sim[b,q,p] = xn[b,q,:] . xn[b,p,:]
    out[b] = -mean_{q,p} sim  =  -(1/Q^2) * || sum_q xn[b,q,:] ||^2

Per batch b (on device):
    sumsq[q] = sum_d x[q,d]^2
    rnorm[q] = sqrt(1/sumsq[q])          (DVE reciprocal + ACT Sqrt)
    s[d]     = sum_q rnorm[q]*x[q,d]     (PE matmul, rnorm as stationary weights)
Host: out[b] = -(1/Q^2) * sum_d s[b,d]^2   (tiny per-core finish)

Final design (v13).  Measured facts that drove it:
  * the load stream runs at HBM line rate; its wall time varies 21-30us
    with cross-core HBM contention (8 data-parallel cores, 2 per stack).
    FEWER SWDGE loads stream faster (per-load bubbles cost ~0.4us each):
    batches 2..13 load as six 1MiB pair loads; b14 per-batch; the
    first/last batches as quarter loads so the first compute starts
    early and the last chain is short.  All loads SWDGE (fp32->bf16 cast
    in DMA); mixing HWDGE x-loads into the stream starves them (a
    0.25MiB HWDGE load once landed 5us late, head-of-line blocking ACT).
  * a load's data is usable ~0.9us after its last byte (DMA sem prop).
  * sumsq: per-element cost is everything.  Big paired ops win: ACT
    Square [128,2048] (~2.0us/pair) -> DVE fold add + segmented reduce
    (~1.8us/pair); two pairs square on DVE (tensor_mul 2x) for balance.
    Per-segment fused ops (DVE scalar_tensor_tensor 467ns / ACT
    Square+accum_out 693ns per [128,256]) cost ~2x per element in fixed
    overheads but have minimal chain latency - used only for batches
    0, 12..15, split across DVE and ACT so tail chains run in parallel
    (b12/b13 put only c3 on ACT: ACT is the tail's critical engine).
    tensor_tensor_reduce wedges the device - never emit it.
  * the gpsimd Q7 is NOT free mid-stream (SWDGE descriptor emission is
    ring-paced until ~4us before stream end): no Pool-engine compute.
  * s cells: batch bb -> PSUM (row 32*(bb%3), col block bb//3); matmul
    out APs may only be based at partitions 0/32/64.  Cell col blocks
    are copied PSUM->SBUF as their last batch finishes (cost = free-dim
    size only; a [1,4096] single-partition copy would cost 4.5us) and
    shipped in 4 DMAs so the final copy+store is tiny.  PSUM is memset
    during the idle preamble (copies read never-written partitions).
    Host unscrambles the cells.
  * cross-engine consumers (fold after ACT square, rnorm after reduce)
    are emitted 1-2 units late so an in-order engine never head-of-line
    blocks on a peer that is still working.
  * post-stream tail ~2us (baseline: ~9-13us); interleaved A/B vs the
    45641ns baseline kernel won 4/4 rounds (medians 50.6 vs 53.6 in a
    contended window, +0.8..+4.0us each round).

Sharding: pure data parallel, batch dim 128 -> 16 per core across 8 cores.
"""

import numpy as np

B, Q, D = 128, 512, 256
N_CORES = 8
BL = B // N_CORES  # 16 batches per core
CHUNKS = 4
EPS = 1e-12

PAIRS = [(2, 3), (4, 5), (6, 7), (8, 9), (10, 11)]
# only the earliest pair squares on DVE: a DVE-square late in the stream
# (e.g. (8,9)) lands its 1.2us mul inside DVE's end-of-stream crunch window,
# while ACT has mid-stream slack to absorb the square instead
DVE_SQUARE_PAIRS = {(2, 3)}


def _cell(bb):
    if bb >= 13:
        return 32 * (bb - 13), 5 * D
    return 32 * (bb % 3), (bb // 3) * D


def _build(nc):
    import concourse.mybir as mybir
    import concourse.tile as tile

    f32 = mybir.dt.float32
    f16 = mybir.dt.bfloat16
    Act = mybir.ActivationFunctionType
    Alu = mybir.AluOpType

    x_d = nc.dram_tensor("x", [BL, Q, D], f32, kind="ExternalInput").ap()
    s_d = nc.dram_tensor("s_out", [3, 6 * D], f32, kind="ExternalOutput").ap()

    with tile.TileContext(nc) as tc:
        with (
            tc.tile_pool(name="xp", bufs=1) as xp,
            tc.tile_pool(name="scr", bufs=1) as scrp,
            tc.tile_pool(name="sqp", bufs=2) as sqp,
            tc.tile_pool(name="small", bufs=1) as small,
            tc.tile_pool(name="fin", bufs=1) as fin,
            tc.tile_pool(name="ps", bufs=1, space="PSUM") as psp,
        ):
            s_ps = psp.tile([96, 6 * D], f32)

            # ---- loads, issued upfront; HWDGE (b0, b1) leads the stream --
            x_tiles = {}  # bb -> tile or (tile_a, tile_b); pairs: (bb0, tile)
            src0 = x_d[0:1].rearrange("b (p c) d -> p b c d", p=128)
            x0a = xp.tile([128, 1, 2, D], f16, tag="x0a")
            x0b = xp.tile([128, 1, 2, D], f16, tag="x0b")
            nc.gpsimd.dma_start(out=x0a[:], in_=src0[:, :, 0:2])
            nc.gpsimd.dma_start(out=x0b[:], in_=src0[:, :, 2:4])
            x_tiles[0] = (x0a, x0b)
            src1 = x_d[1:2].rearrange("b (p c) d -> p b c d", p=128)
            x1 = xp.tile([128, 1, CHUNKS, D], f16, tag="x1")
            nc.gpsimd.dma_start(out=x1[:], in_=src1)
            x_tiles[1] = x1
            for b0p, b1p in PAIRS:
                # one 1MiB load per pair: fewer SWDGE loads = fewer
                # inter-load bubbles (16 small loads cost the stream ~4us).
                # (v6's DMA_15 imbalance came from HWDGE mixing, not this.)
                src = x_d[b0p : b0p + 2].rearrange("b (p c) d -> p b c d", p=128)
                t = xp.tile([128, 2, CHUNKS, D], f16, tag=f"xp{b0p}")
                nc.gpsimd.dma_start(out=t[:], in_=src)
                x_tiles[b0p] = x_tiles[b1p] = ("pair", b0p, t)
            # one pair load for 12+13 (fewer loads = fewer stream bubbles);
            # their compute stays per-batch fused-split for tail latency
            src1213 = x_d[12:14].rearrange("b (p c) d -> p b c d", p=128)
            t1213 = xp.tile([128, 2, CHUNKS, D], f16, tag="xp12")
            nc.gpsimd.dma_start(out=t1213[:], in_=src1213)
            x_tiles[12] = x_tiles[13] = ("pair", 12, t1213)
            src14 = x_d[14:15].rearrange("b (p c) d -> p b c d", p=128)
            x14 = xp.tile([128, 1, CHUNKS, D], f16, tag="x14")
            nc.gpsimd.dma_start(out=x14[:], in_=src14)
            x_tiles[14] = x14
            src15 = x_d[15:16].rearrange("b (p c) d -> p b c d", p=128)
            x15a = xp.tile([128, 1, 2, D], f16, tag="x15a")
            x15b = xp.tile([128, 1, 2, D], f16, tag="x15b")
            nc.gpsimd.dma_start(out=x15a[:], in_=src15[:, :, 0:2])
            nc.gpsimd.dma_start(out=x15b[:], in_=src15[:, :, 2:4])
            x_tiles[15] = (x15a, x15b)

            # zero PSUM in the idle preamble (copies read junk partitions)
            nc.vector.memset(s_ps[:], 0.0)

            # dummy Sqrt pins the ACT table set -> one ACT_TABLE_LOAD
            dummy = small.tile([1, 1], f32, tag="dummy")
            nc.vector.memset(dummy[:], 1.0)
            dummy2 = small.tile([1, 1], f32, tag="dummy2")
            nc.scalar.activation(out=dummy2[:], in_=dummy[:], func=Act.Sqrt)

            scr_d = scrp.tile([128, D], f16, tag="scr_d")
            scr_a = scrp.tile([128, D], f16, tag="scr_a")

            def x_seg(bb, c):
                t = x_tiles[bb]
                if isinstance(t, tuple) and t[0] == "pair":
                    return t[2][:, bb - t[1], c, :]
                if isinstance(t, tuple):
                    return t[c // 2][:, 0, c % 2, :]
                return t[:, 0, c, :]

            # ---- units (arrival order) ----
            # kinds: fsplit (fused, split DVE/ACT), asq (ACT big square ->
            # DVE fold+reduce), dsq (DVE mul -> fold+reduce)
            units = [
                ("fsplit", 0, 0), ("fsplit", 0, 1), ("asq", 1, None),
            ]
            for p in PAIRS:
                units.append(("dsq" if p in DVE_SQUARE_PAIRS else "asq", p, None))
            units += [("fsplit", 12, None), ("fsplit", 13, None),
                      ("fsplit", 14, None), ("fsplit", 15, 0), ("fsplit", 15, 1)]

            # tail rebalance: ACT serializes ~8 fused segs after stream
            # end while DVE has slack -> b12/b13 put only c3 on ACT
            ACT1_TAIL = {12, 13}

            blk_info = {}

            def batches_of(u):
                kind, who, h = units[u]
                return list(who) if isinstance(who, tuple) else [who]

            for u in range(len(units)):
                kind, who, h = units[u]
                segs = []
                for bb in batches_of(u):
                    cs = range(2 * h, 2 * h + 2) if h is not None else range(4)
                    for c in cs:
                        segs.append((bb, c, len(segs)))
                blk_info[u] = dict(segs=segs, is32=False, sumsq=None)

            def get_sumsq(u):
                info = blk_info[u]
                if info["sumsq"] is None:
                    info["sumsq"] = small.tile(
                        [128, len(info["segs"])], f32,
                        tag=f"sumsq{u}", name=f"sumsq{u}",
                    )
                return info["sumsq"]

            def emit_square(u):
                """Emit the square stage; returns deferred work or None."""
                kind, who, h = units[u]
                if kind == "fsplit":
                    bb = who
                    sumsq = get_sumsq(u)
                    cs = range(2 * h, 2 * h + 2) if h is not None else range(4)
                    for c in cs:
                        col = c - 2 * h if h is not None else c
                        acc = sumsq[:, col : col + 1]
                        if (c == 3) if bb in ACT1_TAIL else (c % 2 == 1):
                            nc.scalar.activation(
                                out=scr_a[:], in_=x_seg(bb, c),
                                func=Act.Square, accum_out=acc,
                            )
                        else:
                            nc.vector.scalar_tensor_tensor(
                                out=scr_d[:],
                                in0=x_seg(bb, c), scalar=1.0, in1=x_seg(bb, c),
                                op0=Alu.mult, op1=Alu.mult, accum_out=acc,
                            )
                    return None
                # asq / dsq: big square into a bf16 sq tile
                bbs = batches_of(u)
                n = len(bbs) * CHUNKS * D
                if isinstance(who, tuple):
                    flat = x_tiles[who[0]][2][:].rearrange("p b c d -> p (b c d)")
                else:
                    flat = x_tiles[who][:].rearrange("p b c d -> p (b c d)")
                sq = sqp.tile([128, n], f16, tag="sq", name=f"sq{u}")
                if kind == "asq":
                    nc.scalar.activation(out=sq[:], in_=flat, func=Act.Square)
                else:
                    nc.vector.tensor_mul(sq[:], flat, flat)
                return sq

            def emit_fold_reduce(u, sq):
                sumsq = get_sumsq(u)
                n_seg = len(blk_info[u]["segs"])
                fold = sqp.tile(
                    [128, n_seg, D // 2], f16, tag="fold", name=f"fold{u}"
                )
                sqv = sq[:].rearrange("p (s d) -> p s d", d=D)
                nc.vector.tensor_add(
                    fold[:], sqv[:, :, 0 : D // 2], sqv[:, :, D // 2 : D]
                )
                nc.vector.tensor_reduce(
                    out=sumsq[:],
                    in_=fold[:],
                    axis=mybir.AxisListType.X,
                    op=Alu.add,
                )

            def emit_rnorm_and_mm(u):
                info = blk_info[u]
                n = len(info["segs"])
                wdt = f32 if info["is32"] else f16
                with tc.high_priority():
                    rsum = small.tile([128, n], f32, tag=f"rsum{u}",
                                      name=f"rsum{u}")
                    nc.vector.reciprocal(out=rsum[:], in_=info["sumsq"][:])
                    rnorm = small.tile([128, n], wdt, tag=f"rnorm{u}",
                                       name=f"rnorm{u}")
                    nc.scalar.activation(out=rnorm[:], in_=rsum[:], func=Act.Sqrt)
                for bb, c, col in info["segs"]:
                    r, j = _cell(bb)
                    nc.tensor.matmul(
                        s_ps[r : r + 1, j : j + D],
                        rnorm[:, col : col + 1],
                        x_seg(bb, c),
                        start=(c == 0),
                        stop=(c == CHUNKS - 1),
                    )

            # ---- progressive epilogue ----
            s_sb = fin.tile([96, 6 * D], f32, tag="s_sb")
            rows = s_sb[:].rearrange("(r k) f -> r k f", r=3)[:, 0, :]

            def emit_copy(j, eng):
                c0, c1 = j * D, (j + 1) * D
                if eng == "dve":
                    nc.vector.tensor_copy(s_sb[:, c0:c1], s_ps[:, c0:c1])
                else:
                    nc.scalar.copy(s_sb[:, c0:c1], s_ps[:, c0:c1])

            unit_of_batch = {}
            for u in range(len(units)):
                for bb in batches_of(u):
                    unit_of_batch[bb] = u  # later units win (b0/b15 halves)
            copy_after_unit = {}
            for j, (gate_bb, eng) in enumerate(
                [(2, "dve"), (5, "act"), (8, "dve"), (11, "act"), (12, "dve")]
            ):
                copy_after_unit.setdefault(unit_of_batch[gate_bb], []).append(
                    (j, eng)
                )
            dma_after_j = {1: (0, 512), 3: (512, 1024), 4: (1024, 1280)}

            # ---- emission schedule ----
            # square(u) at u; fold+reduce(u) at u+1; rnorm(u) at u+2 for
            # two-stage units / u+1 for fsplit; copies ride their rnorm.
            fr_at = {}      # unit -> emit fold/reduce at
            rn_at = {}
            sq_of = {}
            for u in range(len(units)):
                kind = units[u][0]
                rn_at[u] = u + (1 if kind == "fsplit" else 2)

            for u in range(len(units)):
                sq = emit_square(u)
                if sq is not None:
                    sq_of[u] = sq
                    fr_at[u] = u + 1
                for v in list(fr_at):
                    if fr_at[v] <= u:
                        emit_fold_reduce(v, sq_of.pop(v))
                        del fr_at[v]
                for v in range(len(units)):
                    if rn_at.get(v) is not None and rn_at[v] <= u:
                        emit_rnorm_and_mm(v)
                        rn_at[v] = None
                        for j, eng in copy_after_unit.get(v, []):
                            emit_copy(j, eng)
                            if j in dma_after_j:
                                a, b = dma_after_j[j]
                                nc.sync.dma_start(
                                    out=s_d[0:3, a:b], in_=rows[:, a:b]
                                )
            for v in list(fr_at):
                emit_fold_reduce(v, sq_of.pop(v))
            for v in range(len(units)):
                if rn_at.get(v) is not None:
                    emit_rnorm_and_mm(v)
                    rn_at[v] = None
                    for j, eng in copy_after_unit.get(v, []):
                        emit_copy(j, eng)
                        if j in dma_after_j:
                            a, b = dma_after_j[j]
                            nc.sync.dma_start(out=s_d[0:3, a:b], in_=rows[:, a:b])

            emit_copy(5, "dve")
            nc.sync.dma_start(out=s_d[0:3, 1280:1536], in_=rows[:, 1280:1536])
    return nc


def _make_nc():
    import concourse.bacc as bacc

    nc = bacc.Bacc(trn_type="TRN2")
    _build(nc)
    nc.finalize()
    return nc


def _finish(s):
    # s: [3, 6*D] cell grid; batch bb at _cell(bb). out[b] = -(1/Q^2)*||s_b||^2
    s = s.astype(np.float32).reshape(3, 6, D)
    v = np.empty((BL, D), np.float32)
    for bb in range(BL):
        r, j = _cell(bb)
        v[bb] = s[r // 32, j // D]
    return -(v * v).sum(axis=-1) / np.float32(Q * Q)


def _run(x, trace=False):
    from concourse.bass_utils import run_bass_kernel_spmd

    in_maps = [
        {"x": np.ascontiguousarray(x[i * BL : (i + 1) * BL])} for i in range(N_CORES)
    ]
    nc = _make_nc()
    res = run_bass_kernel_spmd(
        nc, in_maps, core_ids=list(range(N_CORES)), trace=trace
    )
    out = np.concatenate([_finish(r["s_out"]) for r in res.results], axis=0)
    return out.astype(np.float32), res


def kernel(x: np.ndarray) -> np.ndarray:
    out, _ = _run(np.asarray(x, dtype=np.float32))
    return outFinal design (v13).  Measured facts that drove it:
  * the load stream runs at HBM line rate; its wall time varies 21-30us
    with cross-core HBM contention (8 data-parallel cores, 2 per stack).
    FEWER SWDGE loads stream faster (per-load bubbles cost ~0.4us each):
    batches 2..11 load as five 1MiB pair loads; 12,13,14 per-batch; the
    first/last batches as quarter loads so the first compute starts early
    and the last chain is short.  All loads SWDGE (fp32->bf16 cast in
    DMA); mixing HWDGE x-loads into the stream starves them (a 0.25MiB
    HWDGE load once landed 5us late, head-of-line blocking ACT).
  * a load's data is usable ~0.9us after its last byte (DMA sem prop).
  * sumsq: per-element cost is everything.  Big paired ops win: ACT
    Square [128,2048] (~2.0us/pair) -> DVE fold add + segmented reduce
    (~1.8us/pair); two pairs square on DVE (tensor_mul 2x) for balance.
    Per-segment fused ops (DVE scalar_tensor_tensor 467ns / ACT
    Square+accum_out 693ns per [128,256]) cost ~2x per element in fixed
    overheads but have minimal chain latency - used only for batches
    0, 12..15, split across DVE and ACT so tail chains run in parallel
    (b12/b13 put only c3 on ACT: ACT is the tail's critical engine).
    tensor_tensor_reduce wedges the device - never emit it.
  * the gpsimd Q7 is NOT free mid-stream (SWDGE descriptor emission is
    ring-paced until ~4us before stream end): no Pool-engine compute.
  * s cells: batch bb -> PSUM (row 32*(bb%3), col block bb//3); matmul
    out APs may only be based at partitions 0/32/64.  Cell col blocks
    are copied PSUM->SBUF as their last batch finishes (cost = free-dim
    size only; a [1,4096] single-partition copy would cost 4.5us) and
    shipped in 4 DMAs so the final copy+store is tiny.  PSUM is memset
    during the idle preamble (copies read never-written partitions).
    Host unscrambles the cells.
  * cross-engine consumers (fold after ACT square, rnorm after reduce)
    are emitted 1-2 units late so an in-order engine never head-of-line
    blocks on a peer that is still working.
  * with this structure the post-stream tail is ~2us (baseline: ~9-13us);
    interleaved A/B vs the 45641ns baseline kernel won 4/4 rounds by
    +0.8..+4.0us (medians 50.6 vs 53.6 in a contended window).

Sharding: pure data parallel, batch dim 128 -> 16 per core across 8 cores.
"""

import numpy as np

B, Q, D = 128, 512, 256
N_CORES = 8
BL = B // N_CORES  # 16 batches per core
CHUNKS = 4
EPS = 1e-12

PAIRS = [(2, 3), (4, 5), (6, 7), (8, 9), (10, 11)]
# only the earliest pair squares on DVE: a DVE-square late in the stream
# (e.g. (8,9)) lands its 1.2us mul inside DVE's end-of-stream crunch window,
# while ACT has mid-stream slack to absorb the square instead
DVE_SQUARE_PAIRS = {(2, 3)}


def _cell(bb):
    if bb >= 13:
        return 32 * (bb - 13), 5 * D
    return 32 * (bb % 3), (bb // 3) * D


def _build(nc):
    import concourse.mybir as mybir
    import concourse.tile as tile

    f32 = mybir.dt.float32
    f16 = mybir.dt.bfloat16
    Act = mybir.ActivationFunctionType
    Alu = mybir.AluOpType

    x_d = nc.dram_tensor("x", [BL, Q, D], f32, kind="ExternalInput").ap()
    s_d = nc.dram_tensor("s_out", [3, 6 * D], f32, kind="ExternalOutput").ap()

    with tile.TileContext(nc) as tc:
        with (
            tc.tile_pool(name="xp", bufs=1) as xp,
            tc.tile_pool(name="scr", bufs=1) as scrp,
            tc.tile_pool(name="sqp", bufs=2) as sqp,
            tc.tile_pool(name="small", bufs=1) as small,
            tc.tile_pool(name="fin", bufs=1) as fin,
            tc.tile_pool(name="ps", bufs=1, space="PSUM") as psp,
        ):
            s_ps = psp.tile([96, 6 * D], f32)

            # ---- loads, issued upfront; HWDGE (b0, b1) leads the stream --
            x_tiles = {}  # bb -> tile or (tile_a, tile_b); pairs: (bb0, tile)
            src0 = x_d[0:1].rearrange("b (p c) d -> p b c d", p=128)
            x0a = xp.tile([128, 1, 2, D], f16, tag="x0a")
            x0b = xp.tile([128, 1, 2, D], f16, tag="x0b")
            nc.gpsimd.dma_start(out=x0a[:], in_=src0[:, :, 0:2])
            nc.gpsimd.dma_start(out=x0b[:], in_=src0[:, :, 2:4])
            x_tiles[0] = (x0a, x0b)
            src1 = x_d[1:2].rearrange("b (p c) d -> p b c d", p=128)
            x1 = xp.tile([128, 1, CHUNKS, D], f16, tag="x1")
            nc.gpsimd.dma_start(out=x1[:], in_=src1)
            x_tiles[1] = x1
            for b0p, b1p in PAIRS:
                # one 1MiB load per pair: fewer SWDGE loads = fewer
                # inter-load bubbles (16 small loads cost the stream ~4us).
                # (v6's DMA_15 imbalance came from HWDGE mixing, not this.)
                src = x_d[b0p : b0p + 2].rearrange("b (p c) d -> p b c d", p=128)
                t = xp.tile([128, 2, CHUNKS, D], f16, tag=f"xp{b0p}")
                nc.gpsimd.dma_start(out=t[:], in_=src)
                x_tiles[b0p] = x_tiles[b1p] = ("pair", b0p, t)
            # one pair load for 12+13 (fewer loads = fewer stream bubbles);
            # their compute stays per-batch fused-split for tail latency
            src1213 = x_d[12:14].rearrange("b (p c) d -> p b c d", p=128)
            t1213 = xp.tile([128, 2, CHUNKS, D], f16, tag="xp12")
            nc.gpsimd.dma_start(out=t1213[:], in_=src1213)
            x_tiles[12] = x_tiles[13] = ("pair", 12, t1213)
            src14 = x_d[14:15].rearrange("b (p c) d -> p b c d", p=128)
            x14 = xp.tile([128, 1, CHUNKS, D], f16, tag="x14")
            nc.gpsimd.dma_start(out=x14[:], in_=src14)
            x_tiles[14] = x14
            src15 = x_d[15:16].rearrange("b (p c) d -> p b c d", p=128)
            x15a = xp.tile([128, 1, 2, D], f16, tag="x15a")
            x15b = xp.tile([128, 1, 2, D], f16, tag="x15b")
            nc.gpsimd.dma_start(out=x15a[:], in_=src15[:, :, 0:2])
            nc.gpsimd.dma_start(out=x15b[:], in_=src15[:, :, 2:4])
            x_tiles[15] = (x15a, x15b)

            # zero PSUM in the idle preamble (copies read junk partitions)
            nc.vector.memset(s_ps[:], 0.0)

            # dummy Sqrt pins the ACT table set -> one ACT_TABLE_LOAD
            dummy = small.tile([1, 1], f32, tag="dummy")
            nc.vector.memset(dummy[:], 1.0)
            dummy2 = small.tile([1, 1], f32, tag="dummy2")
            nc.scalar.activation(out=dummy2[:], in_=dummy[:], func=Act.Sqrt)

            scr_d = scrp.tile([128, D], f16, tag="scr_d")
            scr_a = scrp.tile([128, D], f16, tag="scr_a")

            def x_seg(bb, c):
                t = x_tiles[bb]
                if isinstance(t, tuple) and t[0] == "pair":
                    return t[2][:, bb - t[1], c, :]
                if isinstance(t, tuple):
                    return t[c // 2][:, 0, c % 2, :]
                return t[:, 0, c, :]

            # ---- units (arrival order) ----
            # kinds: fsplit (fused, split DVE/ACT), asq (ACT big square ->
            # DVE fold+reduce), dsq (DVE mul -> fold+reduce)
            units = [
                ("fsplit", 0, 0), ("fsplit", 0, 1), ("asq", 1, None),
            ]
            for p in PAIRS:
                units.append(("dsq" if p in DVE_SQUARE_PAIRS else "asq", p, None))
            units += [("fsplit", 12, None), ("fsplit", 13, None),
                      ("fsplit", 14, None), ("fsplit", 15, 0), ("fsplit", 15, 1)]

            # tail rebalance: ACT serializes ~8 fused segs after stream
            # end while DVE has slack -> b12/b13 put only c3 on ACT
            ACT1_TAIL = {12, 13}

            blk_info = {}

            def batches_of(u):
                kind, who, h = units[u]
                return list(who) if isinstance(who, tuple) else [who]

            for u in range(len(units)):
                kind, who, h = units[u]
                segs = []
                for bb in batches_of(u):
                    cs = range(2 * h, 2 * h + 2) if h is not None else range(4)
                    for c in cs:
                        segs.append((bb, c, len(segs)))
                blk_info[u] = dict(segs=segs, is32=False, sumsq=None)

            def get_sumsq(u):
                info = blk_info[u]
                if info["sumsq"] is None:
                    info["sumsq"] = small.tile(
                        [128, len(info["segs"])], f32,
                        tag=f"sumsq{u}", name=f"sumsq{u}",
                    )
                return info["sumsq"]

            def emit_square(u):
                """Emit the square stage; returns deferred work or None."""
                kind, who, h = units[u]
                if kind == "fsplit":
                    bb = who
                    sumsq = get_sumsq(u)
                    cs = range(2 * h, 2 * h + 2) if h is not None else range(4)
                    for c in cs:
                        col = c - 2 * h if h is not None else c
                        acc = sumsq[:, col : col + 1]
                        if (c == 3) if bb in ACT1_TAIL else (c % 2 == 1):
                            nc.scalar.activation(
                                out=scr_a[:], in_=x_seg(bb, c),
                                func=Act.Square, accum_out=acc,
                            )
                        else:
                            nc.vector.scalar_tensor_tensor(
                                out=scr_d[:],
                                in0=x_seg(bb, c), scalar=1.0, in1=x_seg(bb, c),
                                op0=Alu.mult, op1=Alu.mult, accum_out=acc,
                            )
                    return None
                # asq / dsq: big square into a bf16 sq tile
                bbs = batches_of(u)
                n = len(bbs) * CHUNKS * D
                if isinstance(who, tuple):
                    flat = x_tiles[who[0]][2][:].rearrange("p b c d -> p (b c d)")
                else:
                    flat = x_tiles[who][:].rearrange("p b c d -> p (b c d)")
                sq = sqp.tile([128, n], f16, tag="sq", name=f"sq{u}")
                if kind == "asq":
                    nc.scalar.activation(out=sq[:], in_=flat, func=Act.Square)
                else:
                    nc.vector.tensor_mul(sq[:], flat, flat)
                return sq

            def emit_fold_reduce(u, sq):
                sumsq = get_sumsq(u)
                n_seg = len(blk_info[u]["segs"])
                fold = sqp.tile(
                    [128, n_seg, D // 2], f16, tag="fold", name=f"fold{u}"
                )
                sqv = sq[:].rearrange("p (s d) -> p s d", d=D)
                nc.vector.tensor_add(
                    fold[:], sqv[:, :, 0 : D // 2], sqv[:, :, D // 2 : D]
                )
                nc.vector.tensor_reduce(
                    out=sumsq[:],
                    in_=fold[:],
                    axis=mybir.AxisListType.X,
                    op=Alu.add,
                )

            def emit_rnorm_and_mm(u):
                info = blk_info[u]
                n = len(info["segs"])
                wdt = f32 if info["is32"] else f16
                with tc.high_priority():
                    rsum = small.tile([128, n], f32, tag=f"rsum{u}",
                                      name=f"rsum{u}")
                    nc.vector.reciprocal(out=rsum[:], in_=info["sumsq"][:])
                    rnorm = small.tile([128, n], wdt, tag=f"rnorm{u}",
                                       name=f"rnorm{u}")
                    nc.scalar.activation(out=rnorm[:], in_=rsum[:], func=Act.Sqrt)
                for bb, c, col in info["segs"]:
                    r, j = _cell(bb)
                    nc.tensor.matmul(
                        s_ps[r : r + 1, j : j + D],
                        rnorm[:, col : col + 1],
                        x_seg(bb, c),
                        start=(c == 0),
                        stop=(c == CHUNKS - 1),
                    )

            # ---- progressive epilogue ----
            s_sb = fin.tile([96, 6 * D], f32, tag="s_sb")
            rows = s_sb[:].rearrange("(r k) f -> r k f", r=3)[:, 0, :]

            def emit_copy(j, eng):
                c0, c1 = j * D, (j + 1) * D
                if eng == "dve":
                    nc.vector.tensor_copy(s_sb[:, c0:c1], s_ps[:, c0:c1])
                else:
                    nc.scalar.copy(s_sb[:, c0:c1], s_ps[:, c0:c1])

            unit_of_batch = {}
            for u in range(len(units)):
                for bb in batches_of(u):
                    unit_of_batch[bb] = u  # later units win (b0/b15 halves)
            copy_after_unit = {}
            for j, (gate_bb, eng) in enumerate(
                [(2, "dve"), (5, "act"), (8, "dve"), (11, "act"), (12, "dve")]
            ):
                copy_after_unit.setdefault(unit_of_batch[gate_bb], []).append(
                    (j, eng)
                )
            dma_after_j = {1: (0, 512), 3: (512, 1024), 4: (1024, 1280)}

            # ---- emission schedule ----
            # square(u) at u; fold+reduce(u) at u+1; rnorm(u) at u+2 for
            # two-stage units / u+1 for fsplit; copies ride their rnorm.
            fr_at = {}      # unit -> emit fold/reduce at
            rn_at = {}
            sq_of = {}
            for u in range(len(units)):
                kind = units[u][0]
                rn_at[u] = u + (1 if kind == "fsplit" else 2)

            for u in range(len(units)):
                sq = emit_square(u)
                if sq is not None:
                    sq_of[u] = sq
                    fr_at[u] = u + 1
                for v in list(fr_at):
                    if fr_at[v] <= u:
                        emit_fold_reduce(v, sq_of.pop(v))
                        del fr_at[v]
                for v in range(len(units)):
                    if rn_at.get(v) is not None and rn_at[v] <= u:
                        emit_rnorm_and_mm(v)
                        rn_at[v] = None
                        for j, eng in copy_after_unit.get(v, []):
                            emit_copy(j, eng)
                            if j in dma_after_j:
                                a, b = dma_after_j[j]
                                nc.sync.dma_start(
                                    out=s_d[0:3, a:b], in_=rows[:, a:b]
                                )
            for v in list(fr_at):
                emit_fold_reduce(v, sq_of.pop(v))
            for v in range(len(units)):
                if rn_at.get(v) is not None:
                    emit_rnorm_and_mm(v)
                    rn_at[v] = None
                    for j, eng in copy_after_unit.get(v, []):
                        emit_copy(j, eng)
                        if j in dma_after_j:
                            a, b = dma_after_j[j]
                            nc.sync.dma_start(out=s_d[0:3, a:b], in_=rows[:, a:b])

            emit_copy(5, "dve")
            nc.sync.dma_start(out=s_d[0:3, 1280:1536], in_=rows[:, 1280:1536])
    return nc


def _make_nc():
    import concourse.bacc as bacc

    nc = bacc.Bacc(trn_type="TRN2")
    _build(nc)
    nc.finalize()
    return nc


def _finish(s):
    # s: [3, 6*D] cell grid; batch bb at _cell(bb). out[b] = -(1/Q^2)*||s_b||^2
    s = s.astype(np.float32).reshape(3, 6, D)
    v = np.empty((BL, D), np.float32)
    for bb in range(BL):
        r, j = _cell(bb)
        v[bb] = s[r // 32, j // D]
    return -(v * v).sum(axis=-1) / np.float32(Q * Q)


def _run(x, trace=False):
    from concourse.bass_utils import run_bass_kernel_spmd

    in_maps = [
        {"x": np.ascontiguousarray(x[i * BL : (i + 1) * BL])} for i in range(N_CORES)
    ]
    nc = _make_nc()
    res = run_bass_kernel_spmd(
        nc, in_maps, core_ids=list(range(N_CORES)), trace=trace
    )
    out = np.concatenate([_finish(r["s_out"]) for r in res.results], axis=0)
    return out.astype(np.float32), res


def kernel(x: np.ndarray) -> np.ndarray:
    out, _ = _run(np.asarray(x, dtype=np.float32))
    return out
